# revision 1
# baseline (speedup 1.0000x reference)
"""GAT (graph attention) kernel for Trainium2, 8-core SPMD — one head per core.

Reference computation (per head k):
    h = x @ W_k.T + b_k                       # (N, F)
    left[n]  = h[n] . a_left_k ; right[m] = h[m] . a_right_k
    e[n, m]  = leaky_relu(left[n] + right[m], 0.2)
    a        = softmax_m(where(mask[n, m], e, -1e9))
    out_k    = elu(a @ h)                      # (N, F)
Full output = concat_k(out_k)  -> (N, K*F)

Device strategy (per core, attention tiles are [m(partition), n(free)]):
    - hijacked ACT `Exp` table computes exp(leaky_relu(x, 0.2)) in one pass
      (negative-x spline buckets refit to exp(0.2x); positive side untouched,
      so plain exp(v) for v<=0 is recovered with scale=5).
    - project h_T[f, n] = W_k.T.T @ x.T on PE (fp32), bias-add into SBUF
    - left/right via one PE matmul with lhsT = [a_left | a_right]
    - h in [m, f] chunks (lhsT for aggregation) via PE transposes -> bf16
    - main loop over (n-half, m-chunk):
        em  = exp(leaky(left[n] + right[m]))   (one ScalarE inst, bias=right)
        em *= mask                             (VectorE bf16 tensor_tensor, 2x)
        outT[f, n] += h_chunk.T @ em ; sums[n] += ones.T @ em   (PE, PSUM)
    - epilogue: rs = 1/sums, u = outT * rs, elu (exp via scale=5), store [f, n]
    - host transposes out to [n, f] and concatenates heads.

No row-max subtraction is needed: z in [-13, 13] for these input scales.
Masked entries contribute exactly 0 (mask multiply happens after exp).
"""

import json
import os
import shutil
import tempfile

import numpy as np

import concourse.bass as bass
import concourse.tile as tile
from concourse import bacc, mybir
from concourse.bass_utils import run_bass_kernel_spmd
from concourse.masks import make_identity

N_NODES = 4096
F_IN = 512
K_HEADS = 8
F_OUT = 128
NEG_SLOPE = 0.2
N_CORES = 8

f32 = mybir.dt.float32
bf16 = mybir.dt.bfloat16


# --------------------------------------------------------------------------- #
# activation-table hack: make `exp` compute exp(leaky_relu(x, 0.2))
# --------------------------------------------------------------------------- #
def _make_hacked_act_dir(dst):
    from neuronxcc.driver.Job import Job
    from neuronxcc.driver.jobs.support.FindActInfo import findActInfoFile

    src = os.path.dirname(findActInfoFile(Job.getPackageDir(), "gen3"))
    os.makedirs(dst, exist_ok=True)
    for fn in os.listdir(src):
        shutil.copy(os.path.join(src, fn), os.path.join(dst, fn))

    info = json.load(open(os.path.join(dst, "act_info.json")))
    for s in info["act_func_sets"]:
        if "exp" not in s["act"]:
            continue
        prof = json.load(open(os.path.join(dst, s["profile_json"])))
        start = prof["func_to_bkt_start_idx"]["exp"]
        starts = sorted(prof["func_to_bkt_start_idx"].values())
        ends = [e for e in starts if e > start]
        end = ends[0] if ends else prof["bkt_entry_cnt"]

        path = os.path.join(dst, s["bkt_bin"])
        b = np.fromfile(path, dtype=np.float32).reshape(-1, 8).copy()
        sl = b[start:end]
        neg = sl[:, 4] < 0.0
        x0 = sl[neg, 4].astype(np.float64)
        g = np.exp(NEG_SLOPE * x0)
        sl[neg, 0] = g
        sl[neg, 1] = NEG_SLOPE * g
        sl[neg, 2] = NEG_SLOPE**2 * g / 2.0
        sl[neg, 3] = NEG_SLOPE**3 * g / 6.0
        b[start:end] = sl
        b.tofile(path)
    return os.path.join(dst, "act_info.json")


_ACT_DIR = None


def setup_act_tables():
    global _ACT_DIR
    if _ACT_DIR is None:
        d = os.path.join(tempfile.gettempdir(), "gat_act_tables")
        _ACT_DIR = _make_hacked_act_dir(d)
    os.environ["BASS_ACT_ROOT_JSON_PATH"] = _ACT_DIR
    return _ACT_DIR


# --------------------------------------------------------------------------- #
# bass program
# --------------------------------------------------------------------------- #
def build(n_nodes=N_NODES, n_tile=2048, num_devices=N_CORES, timing_mode=False, repeat=1):
    """One head per core. Returns compiled Bacc module.

    timing_mode: large inputs/outputs become Internal DRAM (no host traffic);
    the whole compute body is emitted `repeat` times so device time dominates
    dispatch overhead."""
    setup_act_tables()

    n = n_nodes
    mc_cnt = n // 128          # m-chunks
    halves = n // n_tile       # n-range splits
    cseg = F_IN // 128         # contraction chunks for the projection
    nseg = min(512, n)         # matmul moving-operand segment (PSUM bank)
    tseg = min(512, n_tile)

    nc = bacc.Bacc("TRN2", target_bir_lowering=False, debug=False, num_devices=num_devices)

    big_kind = "Internal" if timing_mode else "ExternalInput"
    xT_d = nc.dram_tensor("xT", [F_IN, n], f32, kind=big_kind).ap()
    wkT_d = nc.dram_tensor("wkT", [F_IN, F_OUT], f32, kind="ExternalInput").ap()
    bk_d = nc.dram_tensor("bk", [F_OUT, 1], f32, kind="ExternalInput").ap()
    alr_d = nc.dram_tensor("alr", [F_OUT, 2], f32, kind="ExternalInput").ap()
    maskT_d = nc.dram_tensor("maskT", [n, n], bf16, kind=big_kind).ap()
    out_kind = "Internal" if timing_mode else "ExternalOutput"
    out_d = nc.dram_tensor("out", [F_OUT, n], f32, kind=out_kind).ap()
    sink_d = None
    if timing_mode:
        sink_d = nc.dram_tensor("sink", [1, 128], f32, kind="ExternalOutput").ap()

    lr_dram = nc.dram_tensor("lr_scratch", [2, n], f32, kind="Internal")
    sums_dram = nc.dram_tensor("sums_scratch", [halves, n_tile], f32, kind="Internal")
    rs_dram = nc.dram_tensor("rs_scratch", [halves, n_tile], f32, kind="Internal")

    def dram_ap(handle, offset, pattern):
        return bass.AP(tensor=handle.ap().tensor, offset=offset, ap=pattern)

    with tile.TileContext(nc) as tc:
        with (
            tc.tile_pool(name="consts", bufs=1) as consts,
            tc.tile_pool(name="work", bufs=3) as work,
            tc.tile_pool(name="epi", bufs=1) as epi,
        ):
            if timing_mode:
                # fill the Internal inputs on-device: x = 0, mask = 1
                fz = consts.tile([128, n], f32, tag="bigbuf")
                nc.vector.memset(fz, 0.0)
                for c in range(cseg):
                    nc.sync.dma_start(out=xT_d[c * 128 : (c + 1) * 128, :], in_=fz)
                fo = consts.tile([128, n], bf16, tag="fo")
                nc.vector.memset(fo, 1.0)
                for r in range(n // 128):
                    nc.sync.dma_start(out=maskT_d[r * 128 : (r + 1) * 128, :], in_=fo)

            emitted_o_sb = [None]
            for _rep in range(repeat):
              # ---------------- phase 0: load constants ---------------- #
              xT_sb = consts.tile([128, cseg, n], f32, tag="bigbuf")
              for c in range(cseg):
                  nc.sync.dma_start(out=xT_sb[:, c, :], in_=xT_d[c * 128 : (c + 1) * 128, :])
              wkT_sb = consts.tile([128, cseg, F_OUT], f32)
              for c in range(cseg):
                  nc.sync.dma_start(out=wkT_sb[:, c, :], in_=wkT_d[c * 128 : (c + 1) * 128, :])
              bk_sb = consts.tile([128, 1], f32)
              nc.sync.dma_start(out=bk_sb, in_=bk_d)
              alr_sb = consts.tile([128, 2], f32)
              nc.sync.dma_start(out=alr_sb, in_=alr_d)
              identity = consts.tile([128, 128], f32)
              make_identity(nc, identity)
              ones_sb = consts.tile([128, 1], bf16)
              nc.vector.memset(ones_sb, 1.0)

              # ---------------- phase 1: h_T = (W_k x.T) + b ---------------- #
              hT_sb = consts.tile([128, n], f32)
              with tc.tile_pool(name="psA", bufs=1, space="PSUM") as psA:
                  hT_ps = psA.tile([128, n], f32, tag="big")
                  for c in range(cseg):
                      for s in range(n // nseg):
                          nc.tensor.matmul(
                              hT_ps[:, s * nseg : (s + 1) * nseg],
                              lhsT=wkT_sb[:, c, :],
                              rhs=xT_sb[:, c, s * nseg : (s + 1) * nseg],
                              start=(c == 0),
                              stop=(c == cseg - 1),
                          )
                  nc.vector.tensor_scalar_add(out=hT_sb, in0=hT_ps, scalar1=bk_sb)

                  # left/right: lr[2, n] = [a_l | a_r].T @ h_T
                  lr_ps = psA.tile([2, n], f32, tag="big")
                  for s in range(n // nseg):
                      nc.tensor.matmul(
                          lr_ps[:, s * nseg : (s + 1) * nseg],
                          lhsT=alr_sb,
                          rhs=hT_sb[:, s * nseg : (s + 1) * nseg],
                          start=True,
                          stop=True,
                      )
                  lr_sb = consts.tile([2, n], f32, tag="bigbuf")
                  nc.vector.tensor_copy(out=lr_sb, in_=lr_ps)
                  nc.sync.dma_start(out=lr_dram.ap(), in_=lr_sb)

              # broadcasts / reshapes of left & right (via DRAM roundtrip)
              left_bc = consts.tile([128, n], f32)
              nc.sync.dma_start(out=left_bc, in_=dram_ap(lr_dram, 0, [[0, 128], [1, n]]))
              right_sc = consts.tile([128, mc_cnt], f32)
              nc.sync.dma_start(
                  out=right_sc, in_=dram_ap(lr_dram, n, [[1, 128], [128, mc_cnt]])
              )

              # ---------------- phase 2: h in [m, f] chunks (bf16) ---------------- #
              h_mf = consts.tile([128, mc_cnt, F_OUT], bf16)
              with tc.tile_pool(name="psB", bufs=4, space="PSUM") as psB:
                  for j in range(mc_cnt):
                      tr_ps = psB.tile([128, 128], f32, tag="tr")
                      nc.tensor.transpose(tr_ps, hT_sb[:, j * 128 : (j + 1) * 128], identity)
                      nc.vector.tensor_copy(out=h_mf[:, j, :], in_=tr_ps)

              # ---------------- phase 3: main attention loop ---------------- #
              with tc.tile_pool(name="psC", bufs=1, space="PSUM") as psC:
                  for half in range(halves):
                      n0 = half * n_tile
                      outT_ps = psC.tile([128, n_tile], f32, tag="outT")
                      sums_ps = psC.tile([1, n_tile], f32, tag="sums")

                      for mc in range(mc_cnt):
                          mask_sb = work.tile([128, n_tile], bf16, tag="mask")
                          nc.sync.dma_start(
                              out=mask_sb,
                              in_=maskT_d[mc * 128 : (mc + 1) * 128, n0 : n0 + n_tile],
                          )
                          # em = exp(leaky(left + right)) in ONE ScalarE pass
                          # (hacked Exp table; bias = per-partition right)
                          em_sb = work.tile([128, n_tile], bf16, tag="em")
                          nc.scalar.activation(
                              out=em_sb,
                              in_=left_bc[:, n0 : n0 + n_tile],
                              func=mybir.ActivationFunctionType.Exp,
                              bias=right_sc[:, mc : mc + 1],
                              scale=1.0,
                          )
                          # em *= mask  (bf16 tensor_tensor, 2x mode, in place)
                          nc.vector.tensor_tensor(
                              out=em_sb, in0=em_sb, in1=mask_sb, op=mybir.AluOpType.mult
                          )
                          first, last = mc == 0, mc == mc_cnt - 1
                          for s in range(n_tile // tseg):
                              nc.tensor.matmul(
                                  outT_ps[:, s * tseg : (s + 1) * tseg],
                                  lhsT=h_mf[:, mc, :],
                                  rhs=em_sb[:, s * tseg : (s + 1) * tseg],
                                  start=first,
                                  stop=last,
                              )
                          for s in range(n_tile // tseg):
                              nc.tensor.matmul(
                                  sums_ps[:, s * tseg : (s + 1) * tseg],
                                  lhsT=ones_sb,
                                  rhs=em_sb[:, s * tseg : (s + 1) * tseg],
                                  start=first,
                                  stop=last,
                              )

                      # ---- epilogue for this half ---- #
                      sums_sb = epi.tile([1, n_tile], f32, tag="sums_sb")
                      nc.vector.tensor_copy(out=sums_sb, in_=sums_ps)
                      nc.sync.dma_start(
                          out=sums_dram.ap()[half : half + 1, :], in_=sums_sb
                      )
                      sums_sc = epi.tile([128, n_tile // 128], f32, tag="sums_sc")
                      nc.sync.dma_start(
                          out=sums_sc,
                          in_=dram_ap(
                              sums_dram, half * n_tile, [[1, 128], [128, n_tile // 128]]
                          ),
                      )
                      rs_sc = epi.tile([128, n_tile // 128], f32, tag="rs_sc")
                      nc.vector.reciprocal(out=rs_sc, in_=sums_sc)
                      nc.sync.dma_start(
                          out=dram_ap(
                              rs_dram, half * n_tile, [[1, 128], [128, n_tile // 128]]
                          ),
                          in_=rs_sc,
                      )
                      rs_bc = epi.tile([128, n_tile], f32, tag="rs_bc")
                      nc.sync.dma_start(
                          out=rs_bc,
                          in_=dram_ap(rs_dram, half * n_tile, [[0, 128], [1, n_tile]]),
                      )
                      # u = outT * rs ; elu(u) = max(u, exp(min(u, 0)) - 1)
                      # (exp of a negative via hacked table: scale=5 recovers exp)
                      u_sb = epi.tile([128, n_tile], f32, tag="u")
                      nc.vector.tensor_tensor(
                          out=u_sb, in0=outT_ps, in1=rs_bc, op=mybir.AluOpType.mult
                      )
                      t_sb = epi.tile([128, n_tile], f32, tag="t")
                      nc.vector.tensor_scalar_min(out=t_sb, in0=u_sb, scalar1=0.0)
                      nc.scalar.activation(
                          out=t_sb,
                          in_=t_sb,
                          func=mybir.ActivationFunctionType.Exp,
                          scale=5.0,
                      )
                      o_sb = epi.tile([128, n_tile], f32, tag="o")
                      nc.vector.scalar_tensor_tensor(
                          out=o_sb,
                          in0=t_sb,
                          scalar=-1.0,
                          in1=u_sb,
                          op0=mybir.AluOpType.add,
                          op1=mybir.AluOpType.max,
                      )
                      nc.sync.dma_start(out=out_d[:, n0 : n0 + n_tile], in_=o_sb)
                      emitted_o_sb[0] = o_sb

            if timing_mode and sink_d is not None:
                nc.sync.dma_start(out=sink_d, in_=emitted_o_sb[0][0:1, 0:128])

    nc.compile()
    return nc


# --------------------------------------------------------------------------- #
# host entry point
# --------------------------------------------------------------------------- #
_NC_CACHE = {}


def _get_nc():
    key = (N_NODES, 2048)
    if key not in _NC_CACHE:
        _NC_CACHE[key] = build(N_NODES, 2048, N_CORES)
    return _NC_CACHE[key]


def make_in_maps(x, mask, W, b, a_left, a_right):
    import ml_dtypes

    xT = np.ascontiguousarray(x.T, dtype=np.float32)
    maskT = np.ascontiguousarray(mask.T).astype(ml_dtypes.bfloat16)
    in_maps = []
    for k in range(K_HEADS):
        Wk = W[k * F_OUT : (k + 1) * F_OUT, :]
        in_maps.append(
            {
                "xT": xT,
                "wkT": np.ascontiguousarray(Wk.T, dtype=np.float32),
                "bk": np.ascontiguousarray(
                    b[k * F_OUT : (k + 1) * F_OUT].reshape(F_OUT, 1), dtype=np.float32
                ),
                "alr": np.ascontiguousarray(
                    np.stack([a_left[k], a_right[k]], axis=1), dtype=np.float32
                ),
                "maskT": maskT,
            }
        )
    return in_maps


def kernel(x, mask, W, b, a_left, a_right):
    x = np.asarray(x)
    mask = np.asarray(mask)
    W = np.asarray(W)
    b = np.asarray(b)
    a_left = np.asarray(a_left)
    a_right = np.asarray(a_right)
    nc = _get_nc()
    in_maps = make_in_maps(x, mask, W, b, a_left, a_right)
    res = run_bass_kernel_spmd(nc, in_maps, core_ids=list(range(N_CORES)))
    outs = [np.ascontiguousarray(res.results[k]["out"].T) for k in range(K_HEADS)]
    return np.concatenate(outs, axis=1)


if __name__ == "__main__":
    import reference as R

    inputs = {k: np.asarray(v) for k, v in R.setup_inputs().items()}
    expected = np.asarray(R.reference(**R.setup_inputs()))
    got = kernel(**inputs)
    aerr = np.abs(got - expected)
    scale = np.abs(expected).max()
    print(f"absmax err {aerr.max():.3e}  scale {scale:.3f}  rel {aerr.max() / scale:.3e}")



# revision 2
# speedup vs baseline: 1.0807x; 1.0807x over previous
"""GAT kernel v2 for Trainium2, 8-core SPMD — one head per core.

Reference computation (per head k):
    h = x @ W_k.T + b_k                       # (N, F)
    l[n] = h[n].a_left_k ; r[m] = h[m].a_right_k
    e[n, m] = leaky_relu(l[n] + r[m], 0.2)
    a       = softmax_m(where(mask[n, m], e, -1e9))
    out_k   = elu(a @ h)                      # (N, F)

Key identity: exp(leaky_relu(z)) = max(e^z, e^{0.2 z}).  Scaling row n of
the attention matrix by s*e^{-0.2 l[n]} (cancels in the softmax):
    em[m, n] = mask * max(EL8[n] * er[m], er2[m])
with EL8 = e^{0.8(l+bl)}, er = s e^{r+br}, er2 = s e^{0.2(r+br)} — fully
separable, so the N x N tile needs NO activation pass: one DVE
tensor_scalar (mult+max against two per-partition scalars, 4x rate)
plus one tensor_tensor mask multiply (f16 2x on DVE; every 4th chunk
on GPSIMD).

l and r come straight from x: l[m] = x[m] @ (W_k.T a_l) + b.a_l — the
l/r matmuls share their stationary operand (an xT slice) with the h
projection, so both land chunk-wise with no transposes and no DRAM
roundtrip.  Softmax sums: PE ones-matmul for k_pe of the 32 m-chunks,
f16 DVE accumulation for the rest (engine balance).  The mask ships
u8 and is consumed directly by the tensor_tensor multiply (mixed
dtype), halving its DMA and SBUF traffic.  Epilogue: u = outT*rs; elu =
max(exp(-relu(-u)) - 1, u) split over ACT and DVE; f16 output.
"""

import os

import numpy as np

import concourse.bass as bass
import concourse.tile as tile
from concourse import bacc, mybir
from concourse.bass_utils import run_bass_kernel_spmd

N_NODES = 4096
F_IN = 512
K_HEADS = 8
F_OUT = 128
N_CORES = 8

# em scale: em = S_EM * e^{...}; cancels in softmax, keeps f16 in range
S_EM = 2.0 ** -7
# m-chunks [0, K_PE) reduce softmax sums on PE; rest accumulate on DVE
K_PE = 26

f32 = mybir.dt.float32
f16 = mybir.dt.float16
bf16 = mybir.dt.bfloat16
u8 = mybir.dt.uint8


def build(
    n_nodes=N_NODES,
    n_tile=1024,
    num_devices=N_CORES,
    timing_mode=False,
    repeat=1,
    k_pe=K_PE,
    mask_mode="u8direct",
    pool_tt=False,
    only="all",
):
    # make sure no stale hacked activation tables leak into this build
    os.environ.pop("BASS_ACT_ROOT_JSON_PATH", None)

    n = n_nodes
    mc_cnt = n // 128          # m-chunks (partition rows of em)
    qcnt = n // n_tile         # n-range quarters
    cseg = F_IN // 128         # contraction chunks for projections
    k_pe = min(k_pe, mc_cnt)

    nc = bacc.Bacc("TRN2", target_bir_lowering=False, debug=False, num_devices=num_devices)

    big_kind = "Internal" if timing_mode else "ExternalInput"
    xT_d = nc.dram_tensor("xT", [F_IN, n], bf16, kind=big_kind).ap()
    wkT_d = nc.dram_tensor("wkT", [F_IN, F_OUT], bf16, kind="ExternalInput").ap()
    wlr_d = nc.dram_tensor("wlr", [F_IN, 2], bf16, kind="ExternalInput").ap()
    bkrow_d = nc.dram_tensor("bkrow", [1, F_OUT], bf16, kind="ExternalInput").ap()
    # params[:, 0] = 0.8*bl ; [:, 1] = br + ln(s) ; [:, 2] = 0.2*br + ln(s)
    params_d = nc.dram_tensor("params", [128, 3], f32, kind="ExternalInput").ap()
    masku8_dram = nc.dram_tensor("masku8", [n, n], u8, kind=big_kind)
    masku8_d = masku8_dram.ap()
    out_kind = "Internal" if timing_mode else "ExternalOutput"
    out_d = nc.dram_tensor("out", [F_OUT, n], f16, kind=out_kind).ap()
    sink_d = None
    if timing_mode:
        sink_d = nc.dram_tensor("sink", [1, 128], f16, kind="ExternalOutput").ap()

    el8_dram = nc.dram_tensor("el8_scratch", [1, n], f16, kind="Internal")
    sums_dram = nc.dram_tensor("sums_scratch", [qcnt, n_tile], f32, kind="Internal")
    rs_dram = nc.dram_tensor("rs_scratch", [qcnt, n_tile], f16, kind="Internal")

    def dram_ap(handle, offset, pattern):
        return bass.AP(tensor=handle.ap().tensor, offset=offset, ap=pattern)

    with tile.TileContext(nc) as tc:
        with (
            tc.tile_pool(name="consts", bufs=1) as consts,
            tc.tile_pool(name="work", bufs=4) as work,
            tc.tile_pool(name="epi", bufs=2) as epi,
            tc.tile_pool(name="pro", bufs=1) as pro,
            tc.tile_pool(name="ps", bufs=1, space="PSUM") as ps,
        ):
            if timing_mode:
                with tc.tile_pool(name="fillp", bufs=1) as fillp:
                    fz = fillp.tile([128, n], bf16, tag="fill")
                    nc.vector.memset(fz, 0.0)
                    for c in range(cseg):
                        nc.sync.dma_start(out=xT_d[c * 128 : (c + 1) * 128, :], in_=fz)
                    fu = fillp.tile([128, n], u8, tag="fillu")
                    nc.vector.memset(fu, 1)
                    for rblk in range(mc_cnt):
                        nc.sync.dma_start(
                            out=masku8_d[rblk * 128 : (rblk + 1) * 128, :], in_=fu
                        )

            emitted_o_sb = [None]
            for _rep in range(repeat):
                # ---------------- constants ---------------- #
                wkT_sb = consts.tile([128, cseg, F_OUT], bf16)
                for c in range(cseg):
                    nc.sync.dma_start(out=wkT_sb[:, c, :], in_=wkT_d[c * 128 : (c + 1) * 128, :])
                wlr_sb = consts.tile([128, cseg, 2], bf16)
                for c in range(cseg):
                    nc.sync.dma_start(out=wlr_sb[:, c, :], in_=wlr_d[c * 128 : (c + 1) * 128, :])
                bkrow_sb = consts.tile([1, F_OUT], bf16)
                nc.sync.dma_start(out=bkrow_sb, in_=bkrow_d)
                params_sb = consts.tile([128, 3], f32)
                nc.sync.dma_start(out=params_sb, in_=params_d)
                ones_f16 = consts.tile([128, 1], f16)
                nc.vector.memset(ones_f16, 1.0)
                onesrow = consts.tile([1, 128], bf16)
                nc.vector.memset(onesrow, 1.0)

                h_mf = consts.tile([128, mc_cnt * F_OUT], f16, tag="hmf_sb", bufs=2)
                EL8_bc = consts.tile([128, n], f16, tag="el8bc", bufs=2)
                er_sc = consts.tile([128, mc_cnt], f32, tag="er", bufs=2)
                er2_sc = consts.tile([128, mc_cnt], f32, tag="er2", bufs=2)

                # ---------------- prologue: projections ---------------- #
                if True:
                    xT_sb = pro.tile([128, cseg, n], bf16, tag="xT")
                    for c in range(cseg):
                        nc.sync.dma_start(
                            out=xT_sb[:, c, :], in_=xT_d[c * 128 : (c + 1) * 128, :]
                        )
                    xT_r = xT_sb
                    wkT_r = wkT_sb
                    wlr_r = wlr_sb

                    if True:
                        # lr per chunk: lrmc[p, 2mc:2mc+2] = [l, r] of node
                        # mc*128+p — shares lhsT (xT slice) with the h proj,
                        # so l/r land directly in per-partition-scalar layout
                        lrmc_ps = ps.tile([128, 2 * mc_cnt], f32, tag="lrmc")
                        # h in [m, f] chunks: h_mf[mc] = x[mc-rows] @ Wk.T (+ b)
                        for b4 in range(mc_cnt // 4):
                            hp = ps.tile([128, 4 * F_OUT], f32, tag="hmf", bufs=1)
                            for j4 in range(4):
                                mc = b4 * 4 + j4
                                o = hp[:, j4 * F_OUT : (j4 + 1) * F_OUT]
                                for c in range(cseg):
                                    nc.tensor.matmul(
                                        o,
                                        lhsT=xT_r[:, c, mc * 128 : (mc + 1) * 128],
                                        rhs=wkT_r[:, c, :],
                                        start=(c == 0),
                                        stop=False,
                                    )
                                    nc.tensor.matmul(
                                        lrmc_ps[:, 2 * mc : 2 * mc + 2],
                                        lhsT=xT_r[:, c, mc * 128 : (mc + 1) * 128],
                                        rhs=wlr_r[:, c, :],
                                        start=(c == 0),
                                        stop=(c == cseg - 1),
                                    )
                                # + ones_col.T @ b_row  (adds bias along f)
                                nc.tensor.matmul(
                                    o,
                                    lhsT=onesrow,
                                    rhs=bkrow_sb,
                                    start=False,
                                    stop=True,
                                )
                            nc.scalar.copy(
                                out=h_mf[:, b4 * 512 : (b4 + 1) * 512], in_=hp
                            )

                        lrmc_sb = pro.tile([128, 2 * mc_cnt], f32, tag="lrmc_sb", bufs=2)
                        nc.vector.tensor_copy(out=lrmc_sb, in_=lrmc_ps)

                    l_in = lrmc_sb[:, 0 : 2 * mc_cnt : 2]
                    r_in = lrmc_sb[:, 1 : 2 * mc_cnt : 2]
                    # EL8 = exp(0.8*l + 0.8*bl) in f16: compute in [128, mc]
                    # layout, roundtrip through DRAM as a row, broadcast
                    el8_mc = pro.tile([128, mc_cnt], f16, tag="el8mc", bufs=2)
                    nc.scalar.activation(
                        out=el8_mc,
                        in_=l_in,
                        func=mybir.ActivationFunctionType.Exp,
                        scale=0.8,
                        bias=params_sb[:, 0:1],
                    )
                    nc.sync.dma_start(
                        out=dram_ap(el8_dram, 0, [[1, 128], [128, mc_cnt]]),
                        in_=el8_mc,
                    )
                    nc.sync.dma_start(
                        out=EL8_bc, in_=dram_ap(el8_dram, 0, [[0, 128], [1, n]])
                    )
                    nc.scalar.activation(
                        out=er_sc,
                        in_=r_in,
                        func=mybir.ActivationFunctionType.Exp,
                        scale=1.0,
                        bias=params_sb[:, 1:2],
                    )
                    nc.scalar.activation(
                        out=er2_sc,
                        in_=r_in,
                        func=mybir.ActivationFunctionType.Exp,
                        scale=0.2,
                        bias=params_sb[:, 2:3],
                    )

                # ---------------- main attention loop ---------------- #
                if True:
                    for q in range(qcnt):
                        n0 = q * n_tile
                        outT_ps = ps.tile([128, n_tile], f32, tag="outT", bufs=2)
                        sums_ps = ps.tile([1, n_tile], f32, tag="sums", bufs=1)
                        acc_sb = None

                        em_const = None
                        if only == "pe":
                            em_const = work.tile([128, n_tile], f16, tag="emc")
                            nc.vector.memset(em_const, 1.0)
                        for mc in range(mc_cnt):
                            if only == "pe":
                                em_sb = em_const
                                first, last = mc == 0, mc == mc_cnt - 1
                                for s in range(n_tile // 512):
                                    seg = slice(s * 512, (s + 1) * 512)
                                    nc.tensor.matmul(
                                        outT_ps[:, seg],
                                        lhsT=h_mf[:, mc * F_OUT : (mc + 1) * F_OUT],
                                        rhs=em_sb[:, seg],
                                        start=first,
                                        stop=last,
                                    )
                                if mc < k_pe:
                                    for s in range(n_tile // 512):
                                        seg = slice(s * 512, (s + 1) * 512)
                                        nc.tensor.matmul(
                                            sums_ps[:, seg],
                                            lhsT=ones_f16,
                                            rhs=em_sb[:, seg],
                                            start=first,
                                            stop=(mc == k_pe - 1),
                                        )
                                continue
                            # mask arrives u8 in DRAM
                            if mask_mode == "cast":
                                # GPSIMD (SWDGE) cast-DMA converts u8->f16
                                # in flight, batched 8 m-chunks per call
                                if mc % 8 == 0:
                                    mbatch = work.tile(
                                        [128, 8, n_tile], f16, tag="mask", bufs=2
                                    )
                                    nc.gpsimd.dma_start(
                                        out=mbatch,
                                        in_=dram_ap(
                                            masku8_dram,
                                            mc * 128 * n + n0,
                                            [[n, 128], [128 * n, 8], [1, n_tile]],
                                        ),
                                    )
                                mask_sb = mbatch[:, mc % 8, :]
                            elif mask_mode == "act":
                                # plain u8 DMA + ACT convert
                                m8 = work.tile([128, n_tile], u8, tag="m8")
                                nc.sync.dma_start(
                                    out=m8,
                                    in_=masku8_d[
                                        mc * 128 : (mc + 1) * 128, n0 : n0 + n_tile
                                    ],
                                )
                                mask_sb = work.tile([128, n_tile], f16, tag="maskc")
                                nc.scalar.copy(out=mask_sb, in_=m8)
                            else:
                                # u8 straight into the TT (mixed dtype)
                                mask_sb = work.tile([128, n_tile], u8, tag="m8d")
                                nc.sync.dma_start(
                                    out=mask_sb,
                                    in_=masku8_d[
                                        mc * 128 : (mc + 1) * 128, n0 : n0 + n_tile
                                    ],
                                )
                            if only == "dma":
                                emitted_o_sb[0] = mbatch
                                continue
                            # v = max(EL8 * er, er2)  — one DVE pass, f16 2x
                            em_sb = work.tile([128, n_tile], f16, tag="em")
                            nc.vector.tensor_scalar(
                                out=em_sb,
                                in0=EL8_bc[:, n0 : n0 + n_tile],
                                scalar1=er_sc[:, mc : mc + 1],
                                scalar2=er2_sc[:, mc : mc + 1],
                                op0=mybir.AluOpType.mult,
                                op1=mybir.AluOpType.max,
                            )
                            # em = v * mask — f16 2x, in place; every 4th
                            # chunk runs on the otherwise-idle GPSIMD engine
                            tt_eng = nc.gpsimd if (pool_tt and mc % 4 == 3) else nc.vector
                            tt_eng.tensor_tensor(
                                out=em_sb, in0=em_sb, in1=mask_sb, op=mybir.AluOpType.mult
                            )
                            first, last = mc == 0, mc == mc_cnt - 1
                            if only in ("all", "pe"):
                              for s in range(n_tile // 512):
                                seg = slice(s * 512, (s + 1) * 512)
                                nc.tensor.matmul(
                                    outT_ps[:, seg],
                                    lhsT=h_mf[:, mc * F_OUT : (mc + 1) * F_OUT],
                                    rhs=em_sb[:, seg],
                                    start=first,
                                    stop=last,
                                )
                            if only not in ("all", "pe"):
                                pass
                            elif mc < k_pe:
                                sums_stop = (k_pe == mc_cnt) and last
                                for s in range(n_tile // 512):
                                    seg = slice(s * 512, (s + 1) * 512)
                                    nc.tensor.matmul(
                                        sums_ps[:, seg],
                                        lhsT=ones_f16,
                                        rhs=em_sb[:, seg],
                                        start=first,
                                        stop=sums_stop,
                                    )
                            else:
                                if mc == k_pe:
                                    acc_sb = work.tile(
                                        [128, n_tile], f16, tag="acc", bufs=2
                                    )
                                    nc.vector.tensor_copy(out=acc_sb, in_=em_sb)
                                else:
                                    with nc.allow_low_precision(
                                        reason="f16 softmax-sum acc, ~0.05% ulp"
                                    ):
                                        nc.vector.tensor_tensor(
                                            out=acc_sb,
                                            in0=acc_sb,
                                            in1=em_sb,
                                            op=mybir.AluOpType.add,
                                        )
                        if only in ("all", "pe") and k_pe < mc_cnt and acc_sb is not None:
                            for s in range(n_tile // 512):
                                seg = slice(s * 512, (s + 1) * 512)
                                nc.tensor.matmul(
                                    sums_ps[:, seg],
                                    lhsT=ones_f16,
                                    rhs=acc_sb[:, seg],
                                    start=False,
                                    stop=True,
                                )

                        if only != "all":
                            emitted_o_sb[0] = em_sb
                            continue
                        # ---- epilogue for this quarter ---- #
                        sums_sb = epi.tile([1, n_tile], f32, tag="sums_sb")
                        nc.scalar.copy(out=sums_sb, in_=sums_ps)
                        nc.sync.dma_start(
                            out=sums_dram.ap()[q : q + 1, :], in_=sums_sb
                        )
                        sums_sc = epi.tile([128, n_tile // 128], f32, tag="sums_sc")
                        nc.sync.dma_start(
                            out=sums_sc,
                            in_=dram_ap(
                                sums_dram, q * n_tile, [[1, 128], [128, n_tile // 128]]
                            ),
                        )
                        rs_sc = epi.tile([128, n_tile // 128], f16, tag="rs_sc")
                        with nc.allow_low_precision(reason="f16 reciprocal of sums"):
                            nc.vector.reciprocal(out=rs_sc, in_=sums_sc)
                        nc.sync.dma_start(
                            out=dram_ap(
                                rs_dram, q * n_tile, [[1, 128], [128, n_tile // 128]]
                            ),
                            in_=rs_sc,
                        )
                        rs_bc = epi.tile([128, n_tile], f16, tag="rs_bc")
                        nc.sync.dma_start(
                            out=rs_bc,
                            in_=dram_ap(rs_dram, q * n_tile, [[0, 128], [1, n_tile]]),
                        )
                        # u = outT * rs ; elu(u) = max(exp(min(u, 0)) - 1, u)
                        u_sb = epi.tile([128, n_tile], f32, tag="u")
                        nc.vector.tensor_tensor(
                            out=u_sb, in0=outT_ps, in1=rs_bc, op=mybir.AluOpType.mult
                        )
                        # t = exp(min(u,0)) = exp(-relu(-u)) — two ACT passes
                        w_sb = epi.tile([128, n_tile], f32, tag="w")
                        nc.scalar.activation(
                            out=w_sb,
                            in_=u_sb,
                            func=mybir.ActivationFunctionType.Relu,
                            scale=-1.0,
                        )
                        nc.scalar.activation(
                            out=w_sb,
                            in_=w_sb,
                            func=mybir.ActivationFunctionType.Exp,
                            scale=-1.0,
                        )
                        # o = max(t - 1, u) in f16
                        o_sb = epi.tile([128, n_tile], f16, tag="o")
                        with nc.allow_low_precision(reason="f16 output"):
                            nc.vector.scalar_tensor_tensor(
                                out=o_sb,
                                in0=w_sb,
                                scalar=-1.0,
                                in1=u_sb,
                                op0=mybir.AluOpType.add,
                                op1=mybir.AluOpType.max,
                            )
                        nc.sync.dma_start(out=out_d[:, n0 : n0 + n_tile], in_=o_sb)
                        emitted_o_sb[0] = o_sb

            if timing_mode and sink_d is not None:
                nc.sync.dma_start(out=sink_d, in_=emitted_o_sb[0][0:1, 0:128])

    nc.compile()
    return nc


# --------------------------------------------------------------------------- #
# host entry point
# --------------------------------------------------------------------------- #
_NC_CACHE = {}


def _get_nc():
    key = (N_NODES, 1024, K_PE)
    if key not in _NC_CACHE:
        _NC_CACHE[key] = build(N_NODES, 1024, N_CORES)
    return _NC_CACHE[key]


def make_in_maps(x, mask, W, b, a_left, a_right):
    import ml_dtypes

    xT = np.ascontiguousarray(x.T).astype(ml_dtypes.bfloat16)
    masku8 = np.ascontiguousarray(mask.T).astype(np.uint8)
    lns = float(np.log(S_EM))
    in_maps = []
    for k in range(K_HEADS):
        Wk = W[k * F_OUT : (k + 1) * F_OUT, :].astype(np.float64)
        bk = b[k * F_OUT : (k + 1) * F_OUT].astype(np.float64)
        al = a_left[k].astype(np.float64)
        ar = a_right[k].astype(np.float64)
        wl = Wk.T @ al
        wr = Wk.T @ ar
        bl = float(bk @ al)
        br = float(bk @ ar)
        params = np.zeros((128, 3), np.float32)
        params[:, 0] = 0.8 * bl
        params[:, 1] = br + lns
        params[:, 2] = 0.2 * br + lns
        m = {
            "xT": xT,
            "wkT": np.ascontiguousarray(Wk.T).astype(ml_dtypes.bfloat16),
            "wlr": np.ascontiguousarray(np.stack([wl, wr], axis=1)).astype(
                ml_dtypes.bfloat16
            ),
            "bkrow": np.ascontiguousarray(bk.reshape(1, F_OUT)).astype(
                ml_dtypes.bfloat16
            ),
            "params": params,
        }
        m["masku8"] = masku8
        in_maps.append(m)
    return in_maps


def kernel(x, mask, W, b, a_left, a_right):
    x = np.asarray(x)
    mask = np.asarray(mask)
    W = np.asarray(W)
    b = np.asarray(b)
    a_left = np.asarray(a_left)
    a_right = np.asarray(a_right)
    nc = _get_nc()
    in_maps = make_in_maps(x, mask, W, b, a_left, a_right)
    res = run_bass_kernel_spmd(nc, in_maps, core_ids=list(range(N_CORES)))
    outs = [
        np.ascontiguousarray(res.results[k]["out"].T.astype(np.float32))
        for k in range(K_HEADS)
    ]
    return np.concatenate(outs, axis=1)


if __name__ == "__main__":
    import reference as R

    inputs = {k: np.asarray(v) for k, v in R.setup_inputs().items()}
    expected = np.asarray(R.reference(**R.setup_inputs()))
    got = kernel(**inputs)
    aerr = np.abs(got - expected)
    scale = np.abs(expected).max()
    print(f"absmax err {aerr.max():.3e}  scale {scale:.3f}  rel {aerr.max() / scale:.3e}")
